# revision 16
# baseline (speedup 1.0000x reference)
"""Causal multi-head attention (B=128, T=256, C=384, H=6, Dh=64) on 8 TRN2
NeuronCores, data-parallel over batch (16 batches per core, no collectives).

v2: transposed-scores formulation — P is never transposed.
  - scores computed directly as ST[ts, tq] = KT_h.T @ QT_h (K stationary), so
    exp(ST) = unnormalized P^T feeds the AV matmul with no transpose
  - causal mask applied by accumulating -1e9 * TRI into the score PSUM via a
    (-1e9*I) stationary matmul before the exp (exp -> exact 0)
  - AV uses the P^T blocks as the *stationary* operand and [V_h | ones] as
    moving, so O lands as [tq, d] with the softmax denominator in column 64
    -> normalization is a per-partition tensor_scalar (no cross-partition
    broadcast needed)
  - O is transposed back to OT [d, tq] on the PE (6 bf16 128x128 transposes
    per batch, ~30x less transpose work than transposing P)
  - output projection consumes OT [D, tq] as stationary; PSUM pressure: one
    shared "big" pool (QK/V/O) + scores + OT + Y = 8 banks exactly
"""

import sys

sys.path.insert(0, "/opt/trn_rl_repo")

import numpy as np
import ml_dtypes

import concourse.bass as bass
import concourse.tile as tile
from concourse import mybir
from concourse.bass_utils import run_bass_kernel_spmd
from concourse.masks import make_identity


def split_multi_waits(nc):
    """This walrus build accepts at most one sync-wait command per
    instruction; hoist extra waits into standalone InstEventSemaphore
    instructions on the same engine queue (queue waits run in order before
    the original instruction, so semantics are preserved)."""
    ctr = [0]

    def mk(engine, wait):
        ctr[0] += 1
        return mybir.InstEventSemaphore(
            name=f"WSPLIT-{ctr[0]}",
            engine=engine,
            ins=[],
            outs=[],
            sync_info=mybir.SyncInfo(on_wait=[wait], on_update=[]),
        )

    for f in nc.m.functions:
        for blk in f.blocks:
            insts = blk.instructions
            out = []
            for inst in insts:
                si = inst.sync_info
                if si is not None and len(si.on_wait) > 1:
                    waits = list(si.on_wait)
                    for w in waits[:-1]:
                        out.append(mk(inst.engine, w))
                    inst.sync_info = mybir.SyncInfo(
                        on_wait=[waits[-1]], on_update=list(si.on_update)
                    )
                out.append(inst)
            insts[:] = out
    return nc


N_CORES = 8
B, T, C = 128, 256, 384
H, DH = 6, 64
BL = B // N_CORES  # batches per core
GB = 2  # batches per projection group
BF16 = mybir.dt.bfloat16
FP32 = mybir.dt.float32
AFT = mybir.ActivationFunctionType
SCALE = DH**-0.5  # 0.125


def build_kernel(bl: int = BL) -> bass.Bass:
    nc = bass.Bass()
    xT = nc.dram_tensor("xT", [bl, C, T], BF16, kind="ExternalInput")
    wqt = nc.dram_tensor("wqt", [C, C], BF16, kind="ExternalInput")  # Wq.T [C, D]
    wkt = nc.dram_tensor("wkt", [C, C], BF16, kind="ExternalInput")
    wvt = nc.dram_tensor("wvt", [C, C], BF16, kind="ExternalInput")
    wot = nc.dram_tensor("wot", [C, C], BF16, kind="ExternalInput")  # Wo.T [D, C]
    y = nc.dram_tensor("y", [bl, T, C], FP32, kind="ExternalOutput")

    with tile.TileContext(nc) as tc:
        with (
            tc.tile_pool(name="const", bufs=1) as const,
            tc.tile_pool(name="xp", bufs=2) as xp,
            tc.tile_pool(name="qkp", bufs=2) as qkp,
            tc.tile_pool(name="vp", bufs=3) as vp,
            tc.tile_pool(name="ptp", bufs=4) as ptp,
            tc.tile_pool(name="osp", bufs=2) as osp,
            tc.tile_pool(name="otp", bufs=2) as otp,
            tc.tile_pool(name="rsp", bufs=4) as rsp,
            tc.tile_pool(name="ysbp", bufs=4) as ysbp,
            tc.tile_pool(name="psB", bufs=4, space="PSUM") as psB,
            tc.tile_pool(name="psS", bufs=4, space="PSUM") as psS,
        ):
            # ---- constants ----
            # causal keep-mask in [ts, tq]: 1 where tq >= ts, else 0 (bf16)
            trik = const.tile([128, 128], BF16)
            nc.gpsimd.memset(trik, 1.0)
            nc.gpsimd.affine_select(
                out=trik, in_=trik, compare_op=mybir.AluOpType.is_ge,
                fill=0.0, base=0, pattern=[[1, 128]], channel_multiplier=-1,
            )
            ident = const.tile([128, 128], BF16)
            make_identity(nc, ident)

            # first batch-group activations and QK weights DMA'd in k-major
            # chunks so the first projection matmul can start after ~3 small
            # transfers instead of all of them
            xt0 = xp.tile([128, 3, GB, T], BF16, tag="xt", name="xt0")
            w_sb = {}
            for name, dram in (("wq", wqt), ("wk", wkt), ("wv", wvt), ("wo", wot)):
                w_sb[name] = const.tile([128, 3, C], BF16, tag=name,
                                        name=f"w_{name}")
            wviews = {n: d.rearrange("(k p) d -> p k d", p=128)
                      for n, d in (("wq", wqt), ("wk", wkt))}
            # issue the x chunks from the (idle) Pool queue in parallel with
            # the weight chunks on the Sync queue — SP's per-issue cost would
            # otherwise serialize the startup critical path
            for k in range(3):
                for bi in range(GB):
                    nc.gpsimd.dma_start(
                        out=xt0[:, k, bi, :],
                        in_=xT[bi].rearrange("(k p) t -> p k t", p=128)[:, k, :],
                    )
            for k in range(3):
                for n in ("wq", "wk"):
                    nc.sync.dma_start(out=w_sb[n][:, k, :],
                                      in_=wviews[n][:, k, :])
            for name, dram in (("wv", wvt), ("wo", wot)):
                nc.sync.dma_start(
                    out=w_sb[name],
                    in_=dram.rearrange("(k p) d -> p k d", p=128))

            n_g = bl // GB
            # deferred work queues: batches whose O-transpose / Y-projection
            # have not been emitted yet (emitted interleaved into later
            # batches' attention so the PE never waits on the norm chain)
            pend_tr = []  # (o_sb, ot_tile, b)
            pend_y = []  # (ot, b)

            def emit_tr(o_sb, ot):
                otp_raw = psB.tile([128, GB * T], FP32, tag="big",
                                   name="otps")
                otp_ps = otp_raw.bitcast(BF16)[:, 0 : 3 * T].rearrange(
                    "p (g t) -> p g t", g=3)
                for dg in range(3):
                    nc.tensor.transpose(
                        otp_ps[:, dg, 0:128],
                        o_sb[:, 0, dg * 128 : (dg + 1) * 128], ident,
                    )
                    nc.tensor.transpose(
                        otp_ps[:, dg, 128:256],
                        o_sb[:, 1, dg * 128 : (dg + 1) * 128], ident,
                    )
                nc.vector.tensor_copy(ot, otp_ps)

            def emit_y_half(ot, b, t2, ys=None):
                if ys is None:
                    ys = psS.tile([128, C], FP32, tag="sc", name=f"ys{t2}")
                for k in range(3):
                    nc.tensor.matmul(
                        ys,
                        lhsT=ot[:, k, t2 * 128 : (t2 + 1) * 128],
                        rhs=w_sb["wo"][:, k, :],
                        start=(k == 0),
                        stop=(k == 2),
                    )
                ysb = ysbp.tile([128, C], FP32, tag="ysb", name=f"ysb{t2}")
                if t2 == 0:
                    nc.scalar.copy(ysb, ys)
                else:
                    nc.vector.tensor_copy(ysb, ys)
                nc.sync.dma_start(
                    out=y[b, t2 * 128 : (t2 + 1) * 128, :], in_=ysb
                )

            for g in range(n_g):
                # ---- load xT for GB batches: [128, k, b, T] ----
                if g == 0:
                    xt = xt0
                else:
                    xt = xp.tile([128, 3, GB, T], BF16, tag="xt")
                    for bi in range(GB):
                        nc.sync.dma_start(
                            out=xt[:, :, bi, :],
                            in_=xT[g * GB + bi].rearrange(
                                "(k p) t -> p k t", p=128),
                        )

                # ---- QT/KT for both batches: [D, b, T] ----
                qt = qkp.tile([128, 3, GB, T], BF16, tag="qt")
                kt = qkp.tile([128, 3, GB, T], BF16, tag="kt")
                # allocate all 6 PSUM tiles up front but run the groups that
                # reuse the previous batch's O accumulators (alloc index 2,3)
                # last, so the PE never waits on the norm chain; this order
                # also produces q0/k0 first, which the first scores need
                qk_ps = [psB.tile([128, GB * T], FP32, tag="big",
                                  name=f"qkps{i}") for i in range(6)]
                order = [(qt, "wq", 0, 0), (kt, "wk", 0, 3),
                         (kt, "wk", 1, 4), (qt, "wq", 1, 1),
                         (qt, "wq", 2, 2), (kt, "wk", 2, 5)]
                for ci, (dst, wname, d, pi) in enumerate(order):
                    w = w_sb[wname]
                    ps = qk_ps[pi]
                    for k in range(3):
                        nc.tensor.matmul(
                            ps,
                            lhsT=w[:, k, d * 128 : (d + 1) * 128],
                            rhs=xt[:, k, :, :],
                            start=(k == 0),
                            stop=(k == 2),
                        )
                    if ci % 2 == 0:
                        nc.scalar.copy(dst[:, d, :, :], ps)
                    else:
                        nc.vector.tensor_copy(dst[:, d, :, :], ps)

                # ---- V = [ts, head, 64|ones] per batch ----
                vs = []
                for bi in range(GB):
                    v = vp.tile([128, 2, H, 65], BF16, tag="v")
                    nc.gpsimd.memset(v[:, :, :, 64:65], 1.0)
                    for t2 in range(2):
                        ps = psB.tile([128, GB * T], FP32, tag="big")
                        for k in range(3):
                            nc.tensor.matmul(
                                ps[:, 0:C],
                                lhsT=xt[:, k, bi, t2 * 128 : (t2 + 1) * 128],
                                rhs=w_sb["wv"][:, k, :],
                                start=(k == 0),
                                stop=(k == 2),
                            )
                        nc.vector.tensor_copy(
                            v[:, t2, :, 0:64],
                            ps[:, 0:C].rearrange("p (h d) -> p h d", h=H),
                        )
                    vs.append(v)

                # ---- attention per batch ----
                for bi in range(GB):
                    b = g * GB + bi
                    v = vs[bi]
                    # O accumulators, one per tq-half: [128, head, 64|sum]
                    op0 = psB.tile([128, GB * T], FP32, tag="big", name="op0")
                    op1 = psB.tile([128, GB * T], FP32, tag="big", name="op1")
                    o0 = op0[:, 0 : H * 65].rearrange("p (h d) -> p h d", h=H)
                    o1 = op1[:, 0 : H * 65].rearrange("p (h d) -> p h d", h=H)
                    pt_tiles = {}

                    def emit_scores(p, bi=bi, pt_tiles=pt_tiles, qt=qt, kt=kt):
                        for sub in range(2):
                            h = 2 * p + sub
                            doff = sub * 64
                            qh = qt[doff : doff + 64, p, bi, :]
                            kh = kt[doff : doff + 64, p, bi, :]
                            # sc cols: 0:256 = ts-grp0 x tq 0:256,
                            #          256:384 = ts-grp1 x tq 128:256
                            sc = psS.tile([128, 384], FP32, tag="sc",
                                          name=f"sc_{h}")
                            nc.tensor.matmul(
                                sc[:, 0:256], lhsT=kh[:, 0:128],
                                rhs=qh[:, 0:256], start=True, stop=True,
                            )
                            nc.tensor.matmul(
                                sc[:, 256:384], lhsT=kh[:, 128:256],
                                rhs=qh[:, 128:256], start=True, stop=True,
                            )
                            pt = ptp.tile([128, 384], BF16, tag="pt",
                                          name=f"pt_{h}")
                            nc.scalar.activation(pt, sc, AFT.Exp, scale=SCALE)
                            # zero both causally-masked diagonal blocks in
                            # one op (stepped AP); even head on DVE (its AV
                            # comes first), odd head on the idle Pool engine
                            ptd = pt.rearrange("p (a b) -> p a b", a=3)[:, 0::2, :]
                            trikb = trik.unsqueeze(1).broadcast_to([128, 2, 128])
                            if sub == 0:
                                nc.vector.tensor_mul(ptd, ptd, trikb)
                            else:
                                nc.gpsimd.tensor_mul(ptd, ptd, trikb)
                            pt_tiles[h] = pt

                    def emit_av(p, v=v, o0=o0, o1=o1, pt_tiles=pt_tiles):
                        # within each head: unmasked full block first (needs
                        # only the exp), then the mask-dependent diagonals
                        for sub in range(2):
                            h = 2 * p + sub
                            pt = pt_tiles[h]
                            nc.tensor.matmul(
                                o1[:, h, :], lhsT=pt[:, 128:256],
                                rhs=v[:, 0, h, :], start=True, stop=False,
                            )
                            nc.tensor.matmul(
                                o0[:, h, :], lhsT=pt[:, 0:128],
                                rhs=v[:, 0, h, :], start=True, stop=True,
                            )
                            nc.tensor.matmul(
                                o1[:, h, :], lhsT=pt[:, 256:384],
                                rhs=v[:, 1, h, :], start=False, stop=True,
                            )

                    # normalized O in SBUF [tq-half, D] bf16
                    o_sb = osp.tile([128, 2, C], BF16, tag="osb")

                    def emit_norm(o_sb=o_sb, o0=o0, o1=o1):
                        for half, op_ in ((0, o0), (1, o1)):
                            rs = rsp.tile([128, H], FP32, tag="rs",
                                          name=f"rs{half}")
                            nc.vector.reciprocal(rs, op_[:, :, 64:65])
                            rsb = rs.unsqueeze(-1).broadcast_to([128, H, 64])
                            dst = o_sb[:, half, :].rearrange(
                                "p (h d) -> p h d", h=H)
                            nc.vector.tensor_mul(dst, op_[:, :, 0:64], rsb)

                    emit_scores(0)
                    emit_scores(1)
                    # fill the exp(h0)->mask latency with the previous
                    # batch's O transposes (no pending dependencies)
                    if pend_tr:
                        o_prev, ot_prev, b_prev = pend_tr.pop(0)
                        emit_tr(o_prev, ot_prev)
                        pend_y.append((ot_prev, b_prev))
                    emit_av(0)
                    emit_scores(2)
                    if pend_y:
                        emit_y_half(*pend_y[0], 0)
                    emit_av(1)
                    if pend_y:
                        emit_y_half(*pend_y.pop(0), 1)
                    emit_av(2)
                    emit_norm()
                    ot_t = otp.tile([128, 3, T], BF16, tag="ot")
                    pend_tr.append((o_sb, ot_t, b))

            # drain deferred work: transpose first so its Y overlaps the
            # earlier batch's Y chain
            while pend_tr:
                o_prev, ot_prev, b_prev = pend_tr.pop(0)
                emit_tr(o_prev, ot_prev)
                pend_y.append((ot_prev, b_prev))
            while pend_y:
                ot_, b_ = pend_y.pop(0)
                emit_y_half(ot_, b_, 0)
                emit_y_half(ot_, b_, 1)
    return nc


_NC = None


def _get_nc():
    global _NC
    if _NC is None:
        _NC = split_multi_waits(build_kernel())
    return _NC


def kernel(x, Wq, Wk, Wv, Wo, _trace=False):
    bf16 = ml_dtypes.bfloat16
    wq_t = np.ascontiguousarray(Wq.T).astype(bf16)
    wk_t = np.ascontiguousarray(Wk.T).astype(bf16)
    wv_t = np.ascontiguousarray(Wv.T).astype(bf16)
    wo_t = np.ascontiguousarray(Wo.T).astype(bf16)
    in_maps = []
    for i in range(N_CORES):
        xs = x[i * BL : (i + 1) * BL]  # [BL, T, C]
        xs_t = np.ascontiguousarray(xs.transpose(0, 2, 1)).astype(bf16)
        in_maps.append(
            {"xT": xs_t, "wqt": wq_t, "wkt": wk_t, "wvt": wv_t, "wot": wo_t}
        )
    res = run_bass_kernel_spmd(
        _get_nc(), in_maps, list(range(N_CORES)), trace=_trace
    )
    out = np.concatenate([r["y"] for r in res.results], axis=0)
    if _trace:
        return out.astype(np.float32), res
    return out.astype(np.float32)


# revision 17
# speedup vs baseline: 1.0073x; 1.0073x over previous
"""Causal multi-head attention (B=128, T=256, C=384, H=6, Dh=64) on 8 TRN2
NeuronCores, data-parallel over batch (16 batches per core, no collectives).

Design (vs. a naive port): the kernel is PE-streaming-bound, so everything
is organized to keep the 128x128 PE array fed back-to-back (it only reaches
2.4 GHz after ~3us of gap-free execution; every stall also costs a p-state
re-ramp):
  - scores are computed pre-transposed, ST[ts, tq] = KT_h.T @ QT_h with K
    stationary, so exp(ST) = unnormalized P^T feeds AV with no transposes
    (the baseline spent ~54us transposing P on the PE)
  - causal masking multiplies the two 128x128 diagonal blocks of exp(ST)
    by a 0/1 triangle on DVE (even heads) / GPSIMD (odd heads), off the PE
  - AV uses the P^T blocks as stationary and [V_h | ones] as moving, so
    O lands as [tq, d] with the softmax denominator in column 64;
    normalization is then a cheap per-partition reciprocal + one
    broadcast_to multiply per tq-half, fused with the PSUM->SBUF copy
  - O is transposed back to OT [d, tq] on the PE (6 bf16 128x128
    transposes per batch) for the output projection
  - per-batch schedule interleaves deferred work into dependency bubbles:
    scores(p0,p1) | O-transposes(prev batch) | AV(p0) | scores(p2) |
    Y-half(prev) | AV(p1) | Y-half(prev) | AV(p2); Y PSUM tiles share the
    scores pool, O/QK/V/transpose PSUM share one 4-bank pool (8 banks total)
  - QK projection PSUM groups are consumed in an order that avoids reusing
    the banks still held by the previous batch's O accumulators
  - first x chunks are DMA'd from the Pool queue in parallel with weights
    on the Sync queue to cut the startup serial-issue chain
"""

import sys

sys.path.insert(0, "/opt/trn_rl_repo")

import numpy as np
import ml_dtypes

import concourse.bass as bass
import concourse.tile as tile
from concourse import mybir
from concourse.bass_utils import run_bass_kernel_spmd
from concourse.masks import make_identity


def split_multi_waits(nc):
    """This walrus build accepts at most one sync-wait command per
    instruction; hoist extra waits into standalone InstEventSemaphore
    instructions on the same engine queue (queue waits run in order before
    the original instruction, so semantics are preserved)."""
    ctr = [0]

    def mk(engine, wait):
        ctr[0] += 1
        return mybir.InstEventSemaphore(
            name=f"WSPLIT-{ctr[0]}",
            engine=engine,
            ins=[],
            outs=[],
            sync_info=mybir.SyncInfo(on_wait=[wait], on_update=[]),
        )

    for f in nc.m.functions:
        for blk in f.blocks:
            insts = blk.instructions
            out = []
            for inst in insts:
                si = inst.sync_info
                if si is not None and len(si.on_wait) > 1:
                    waits = list(si.on_wait)
                    for w in waits[:-1]:
                        out.append(mk(inst.engine, w))
                    inst.sync_info = mybir.SyncInfo(
                        on_wait=[waits[-1]], on_update=list(si.on_update)
                    )
                out.append(inst)
            insts[:] = out
    return nc


N_CORES = 8
B, T, C = 128, 256, 384
H, DH = 6, 64
BL = B // N_CORES  # batches per core
GB = 2  # batches per projection group
BF16 = mybir.dt.bfloat16
FP32 = mybir.dt.float32
AFT = mybir.ActivationFunctionType
SCALE = DH**-0.5  # 0.125


def build_kernel(bl: int = BL) -> bass.Bass:
    nc = bass.Bass()
    xT = nc.dram_tensor("xT", [bl, C, T], BF16, kind="ExternalInput")
    wqt = nc.dram_tensor("wqt", [C, C], BF16, kind="ExternalInput")  # Wq.T [C, D]
    wkt = nc.dram_tensor("wkt", [C, C], BF16, kind="ExternalInput")
    wvt = nc.dram_tensor("wvt", [C, C], BF16, kind="ExternalInput")
    wot = nc.dram_tensor("wot", [C, C], BF16, kind="ExternalInput")  # Wo.T [D, C]
    y = nc.dram_tensor("y", [bl, T, C], FP32, kind="ExternalOutput")

    with tile.TileContext(nc) as tc:
        with (
            tc.tile_pool(name="const", bufs=1) as const,
            tc.tile_pool(name="xp", bufs=2) as xp,
            tc.tile_pool(name="qkp", bufs=2) as qkp,
            tc.tile_pool(name="vp", bufs=3) as vp,
            tc.tile_pool(name="ptp", bufs=4) as ptp,
            tc.tile_pool(name="osp", bufs=2) as osp,
            tc.tile_pool(name="otp", bufs=2) as otp,
            tc.tile_pool(name="rsp", bufs=4) as rsp,
            tc.tile_pool(name="ysbp", bufs=4) as ysbp,
            tc.tile_pool(name="psB", bufs=4, space="PSUM") as psB,
            tc.tile_pool(name="psS", bufs=4, space="PSUM") as psS,
        ):
            # ---- constants ----
            # causal keep-mask in [ts, tq]: 1 where tq >= ts, else 0 (bf16)
            trik = const.tile([128, 128], BF16)
            nc.gpsimd.memset(trik, 1.0)
            nc.gpsimd.affine_select(
                out=trik, in_=trik, compare_op=mybir.AluOpType.is_ge,
                fill=0.0, base=0, pattern=[[1, 128]], channel_multiplier=-1,
            )
            ident = const.tile([128, 128], BF16)
            make_identity(nc, ident)

            # first batch-group activations and QK weights DMA'd in k-major
            # chunks so the first projection matmul can start after ~3 small
            # transfers instead of all of them
            xt0 = xp.tile([128, 3, GB, T], BF16, tag="xt", name="xt0")
            w_sb = {}
            for name, dram in (("wq", wqt), ("wk", wkt), ("wv", wvt), ("wo", wot)):
                w_sb[name] = const.tile([128, 3, C], BF16, tag=name,
                                        name=f"w_{name}")
            wviews = {n: d.rearrange("(k p) d -> p k d", p=128)
                      for n, d in (("wq", wqt), ("wk", wkt))}
            # issue the x chunks from the (idle) Pool queue in parallel with
            # the weight chunks on the Sync queue — SP's per-issue cost would
            # otherwise serialize the startup critical path
            for k in range(3):
                for bi in range(GB):
                    nc.gpsimd.dma_start(
                        out=xt0[:, k, bi, :],
                        in_=xT[bi].rearrange("(k p) t -> p k t", p=128)[:, k, :],
                    )
            for k in range(3):
                for n in ("wq", "wk"):
                    nc.sync.dma_start(out=w_sb[n][:, k, :],
                                      in_=wviews[n][:, k, :])
            for name, dram in (("wv", wvt), ("wo", wot)):
                nc.sync.dma_start(
                    out=w_sb[name],
                    in_=dram.rearrange("(k p) d -> p k d", p=128))

            n_g = bl // GB
            # deferred work queues: batches whose O-transpose / Y-projection
            # have not been emitted yet (emitted interleaved into later
            # batches' attention so the PE never waits on the norm chain)
            pend_tr = []  # (o_sb, ot_tile, b)
            pend_y = []  # (ot, b)

            def emit_tr(o_sb, ot):
                otp_raw = psB.tile([128, GB * T], FP32, tag="big",
                                   name="otps")
                otp_ps = otp_raw.bitcast(BF16)[:, 0 : 3 * T].rearrange(
                    "p (g t) -> p g t", g=3)
                for dg in range(3):
                    nc.tensor.transpose(
                        otp_ps[:, dg, 0:128],
                        o_sb[:, 0, dg * 128 : (dg + 1) * 128], ident,
                    )
                    nc.tensor.transpose(
                        otp_ps[:, dg, 128:256],
                        o_sb[:, 1, dg * 128 : (dg + 1) * 128], ident,
                    )
                nc.vector.tensor_copy(ot, otp_ps)

            def emit_y_half(ot, b, t2, ys=None):
                if ys is None:
                    ys = psS.tile([128, C], FP32, tag="sc", name=f"ys{t2}")
                for k in range(3):
                    nc.tensor.matmul(
                        ys,
                        lhsT=ot[:, k, t2 * 128 : (t2 + 1) * 128],
                        rhs=w_sb["wo"][:, k, :],
                        start=(k == 0),
                        stop=(k == 2),
                    )
                ysb = ysbp.tile([128, C], FP32, tag="ysb", name=f"ysb{t2}")
                if t2 == 0:
                    nc.scalar.copy(ysb, ys)
                else:
                    nc.vector.tensor_copy(ysb, ys)
                nc.sync.dma_start(
                    out=y[b, t2 * 128 : (t2 + 1) * 128, :], in_=ysb
                )

            for g in range(n_g):
                # ---- load xT for GB batches: [128, k, b, T] ----
                if g == 0:
                    xt = xt0
                else:
                    xt = xp.tile([128, 3, GB, T], BF16, tag="xt")
                    for bi in range(GB):
                        nc.sync.dma_start(
                            out=xt[:, :, bi, :],
                            in_=xT[g * GB + bi].rearrange(
                                "(k p) t -> p k t", p=128),
                        )

                # ---- QT/KT for both batches: [D, b, T] ----
                qt = qkp.tile([128, 3, GB, T], BF16, tag="qt")
                kt = qkp.tile([128, 3, GB, T], BF16, tag="kt")
                # allocate all 6 PSUM tiles up front but run the groups that
                # reuse the previous batch's O accumulators (alloc index 2,3)
                # last, so the PE never waits on the norm chain; this order
                # also produces q0/k0 first, which the first scores need
                qk_ps = [psB.tile([128, GB * T], FP32, tag="big",
                                  name=f"qkps{i}") for i in range(6)]
                order = [(qt, "wq", 0, 0), (kt, "wk", 0, 3),
                         (kt, "wk", 1, 4), (qt, "wq", 1, 1),
                         (qt, "wq", 2, 2), (kt, "wk", 2, 5)]
                for ci, (dst, wname, d, pi) in enumerate(order):
                    w = w_sb[wname]
                    ps = qk_ps[pi]
                    for k in range(3):
                        nc.tensor.matmul(
                            ps,
                            lhsT=w[:, k, d * 128 : (d + 1) * 128],
                            rhs=xt[:, k, :, :],
                            start=(k == 0),
                            stop=(k == 2),
                        )
                    if ci % 2 == 0:
                        nc.scalar.copy(dst[:, d, :, :], ps)
                    else:
                        nc.vector.tensor_copy(dst[:, d, :, :], ps)

                # ---- V = [ts, head, 64|ones] per batch ----
                vs = []
                for bi in range(GB):
                    v = vp.tile([128, 2, H, 65], BF16, tag="v")
                    nc.gpsimd.memset(v[:, :, :, 64:65], 1.0)
                    for t2 in range(2):
                        ps = psB.tile([128, GB * T], FP32, tag="big")
                        for k in range(3):
                            nc.tensor.matmul(
                                ps[:, 0:C],
                                lhsT=xt[:, k, bi, t2 * 128 : (t2 + 1) * 128],
                                rhs=w_sb["wv"][:, k, :],
                                start=(k == 0),
                                stop=(k == 2),
                            )
                        nc.vector.tensor_copy(
                            v[:, t2, :, 0:64],
                            ps[:, 0:C].rearrange("p (h d) -> p h d", h=H),
                        )
                    vs.append(v)

                # ---- attention per batch ----
                for bi in range(GB):
                    b = g * GB + bi
                    v = vs[bi]
                    # O accumulators, one per tq-half: [128, head, 64|sum]
                    op0 = psB.tile([128, GB * T], FP32, tag="big", name="op0")
                    op1 = psB.tile([128, GB * T], FP32, tag="big", name="op1")
                    o0 = op0[:, 0 : H * 65].rearrange("p (h d) -> p h d", h=H)
                    o1 = op1[:, 0 : H * 65].rearrange("p (h d) -> p h d", h=H)
                    pt_tiles = {}

                    def emit_scores(p, bi=bi, pt_tiles=pt_tiles, qt=qt, kt=kt):
                        for sub in range(2):
                            h = 2 * p + sub
                            doff = sub * 64
                            qh = qt[doff : doff + 64, p, bi, :]
                            kh = kt[doff : doff + 64, p, bi, :]
                            # sc cols: 0:256 = ts-grp0 x tq 0:256,
                            #          256:384 = ts-grp1 x tq 128:256
                            sc = psS.tile([128, 384], FP32, tag="sc",
                                          name=f"sc_{h}")
                            nc.tensor.matmul(
                                sc[:, 0:256], lhsT=kh[:, 0:128],
                                rhs=qh[:, 0:256], start=True, stop=True,
                            )
                            nc.tensor.matmul(
                                sc[:, 256:384], lhsT=kh[:, 128:256],
                                rhs=qh[:, 128:256], start=True, stop=True,
                            )
                            pt = ptp.tile([128, 384], BF16, tag="pt",
                                          name=f"pt_{h}")
                            nc.scalar.activation(pt, sc, AFT.Exp, scale=SCALE)
                            # zero both causally-masked diagonal blocks in
                            # one op (stepped AP); even head on DVE (its AV
                            # comes first), odd head on the idle Pool engine
                            ptd = pt.rearrange("p (a b) -> p a b", a=3)[:, 0::2, :]
                            trikb = trik.unsqueeze(1).broadcast_to([128, 2, 128])
                            if sub == 0:
                                nc.vector.tensor_mul(ptd, ptd, trikb)
                            else:
                                nc.gpsimd.tensor_mul(ptd, ptd, trikb)
                            pt_tiles[h] = pt

                    def emit_av(p, v=v, o0=o0, o1=o1, pt_tiles=pt_tiles):
                        # within each head: unmasked full block first (needs
                        # only the exp), then the mask-dependent diagonals
                        for sub in range(2):
                            h = 2 * p + sub
                            pt = pt_tiles[h]
                            nc.tensor.matmul(
                                o1[:, h, :], lhsT=pt[:, 128:256],
                                rhs=v[:, 0, h, :], start=True, stop=False,
                            )
                            nc.tensor.matmul(
                                o0[:, h, :], lhsT=pt[:, 0:128],
                                rhs=v[:, 0, h, :], start=True, stop=True,
                            )
                            nc.tensor.matmul(
                                o1[:, h, :], lhsT=pt[:, 256:384],
                                rhs=v[:, 1, h, :], start=False, stop=True,
                            )

                    # normalized O in SBUF [tq-half, D] bf16
                    o_sb = osp.tile([128, 2, C], BF16, tag="osb")

                    def emit_norm(o_sb=o_sb, o0=o0, o1=o1):
                        for half, op_ in ((0, o0), (1, o1)):
                            rs = rsp.tile([128, H], FP32, tag="rs",
                                          name=f"rs{half}")
                            nc.vector.reciprocal(rs, op_[:, :, 64:65])
                            rsb = rs.unsqueeze(-1).broadcast_to([128, H, 64])
                            dst = o_sb[:, half, :].rearrange(
                                "p (h d) -> p h d", h=H)
                            nc.vector.tensor_mul(dst, op_[:, :, 0:64], rsb)

                    emit_scores(0)
                    emit_scores(1)
                    # fill the exp(h0)->mask latency with the previous
                    # batch's O transposes (no pending dependencies)
                    if pend_tr:
                        o_prev, ot_prev, b_prev = pend_tr.pop(0)
                        emit_tr(o_prev, ot_prev)
                        pend_y.append((ot_prev, b_prev))
                    emit_av(0)
                    emit_scores(2)
                    if pend_y:
                        emit_y_half(*pend_y[0], 0)
                    emit_av(1)
                    if pend_y:
                        emit_y_half(*pend_y.pop(0), 1)
                    emit_av(2)
                    emit_norm()
                    ot_t = otp.tile([128, 3, T], BF16, tag="ot")
                    pend_tr.append((o_sb, ot_t, b))

            # drain deferred work: transpose first so its Y overlaps the
            # earlier batch's Y chain
            while pend_tr:
                o_prev, ot_prev, b_prev = pend_tr.pop(0)
                emit_tr(o_prev, ot_prev)
                pend_y.append((ot_prev, b_prev))
            while pend_y:
                ot_, b_ = pend_y.pop(0)
                emit_y_half(ot_, b_, 0)
                emit_y_half(ot_, b_, 1)
    return nc


_NC = None


def _get_nc():
    global _NC
    if _NC is None:
        _NC = split_multi_waits(build_kernel())
    return _NC


def kernel(x, Wq, Wk, Wv, Wo, _trace=False):
    bf16 = ml_dtypes.bfloat16
    wq_t = np.ascontiguousarray(Wq.T).astype(bf16)
    wk_t = np.ascontiguousarray(Wk.T).astype(bf16)
    wv_t = np.ascontiguousarray(Wv.T).astype(bf16)
    wo_t = np.ascontiguousarray(Wo.T).astype(bf16)
    in_maps = []
    for i in range(N_CORES):
        xs = x[i * BL : (i + 1) * BL]  # [BL, T, C]
        xs_t = np.ascontiguousarray(xs.transpose(0, 2, 1)).astype(bf16)
        in_maps.append(
            {"xT": xs_t, "wqt": wq_t, "wkt": wk_t, "wvt": wv_t, "wot": wo_t}
        )
    res = run_bass_kernel_spmd(
        _get_nc(), in_maps, list(range(N_CORES)), trace=_trace
    )
    out = np.concatenate([r["y"] for r in res.results], axis=0)
    if _trace:
        return out.astype(np.float32), res
    return out.astype(np.float32)
